# revision 12
# baseline (speedup 1.0000x reference)
"""Trainium2 Bass kernel for nn_NeuralAdaptation (auditory-nerve synapse adaptation).

Math (per lane = (batch, channel)):
    ppi_t = (PIREST/ln2) * softplus(p1 * ihcl_t)
    CI_t  = A_t * CI_{t-1} + beta * CL_{t-1},   A_t = (1 - a_i*PL) - a_i*ppi_t
    CL_t  = gamma * CI_t + delta * CL_{t-1} + c
    out_t = CI_t * ppi_t        (out_0 = SPONT)

Strategy: 8-way data parallel over lanes (1280 lanes -> 160/core).  The coupled
2-state linear recurrence is solved with DVE `tensor_tensor_scan` instructions
(one scalar first-order recurrence per partition at 1 elem/cycle) using
Gauss-Seidel fixed-point iteration (contraction ~0.107/round, R rounds + final
CI pass).  Per core: 128 "primary" lanes sit one-per-partition with time along
the free dim; the remaining 32 lanes are split into two 5000-step segments
([64,5000] region) whose segment-1 initial states lag one round behind.
Time is processed in two sequential halves of 10000 steps to fit SBUF.
"""
import math
import os
import sys

for _p in ("/opt/trn_rl_repo", "/root/.axon_site/_ro/trn_rl_repo"):
    if os.path.isdir(_p) and _p not in sys.path:
        sys.path.insert(0, _p)

import numpy as np
import concourse.bacc as bacc
import concourse.mybir as mybir
from concourse.tile import TileContext
from concourse.bass_utils import run_bass_kernel_spmd

# ---------------- model constants ----------------
SR = 20000.0
VI = 0.0005
VL = 0.005
PG = 0.03
PL = 0.06
PIREST = 0.012
PIMAX = 0.6
SPONT = 50.0

LN2 = math.log(2.0)
P1 = math.log(math.exp(LN2 * (PIMAX / PIREST)) - 1.0)   # ~34.6573590
PSC = PIREST / LN2                                      # p3/p1
A_I = (1.0 / SR) / VI                                   # 0.1
A_L = (1.0 / SR) / VL                                   # 0.01
C1 = 1.0 - A_I * PL                                     # 0.994  (A = C1 + C2*softplus)
C2 = -A_I * PSC
BETA = A_I * PL                                         # 0.006
DELTA = 1.0 - A_L * PL - A_L * PG                       # 0.9991
GAMMA = A_L * PL                                        # 0.0006
CI0 = SPONT / PIREST                                    # 4166.6667
CL0 = CI0 * ((PIREST + PL) / PL)                        # 5000
CG = CL0 * (1.0 + PL / PG) - CI0 * (PL / PG)            # 6666.6667
CCONST = A_L * PG * CG                                  # 2.0
GB = BETA * GAMMA                                       # 3.6e-6   d = GB*CI + BC
BC = BETA * CCONST                                      # 0.012
CL0B = CL0 * BETA                                       # 30.0
VINIT = (CL0B - GB * CI0 - BC) / DELTA                  # virtual delta-scan init for t=0
PPIA = -1.0 / A_I                                       # ppi = PPIA*A + PPIB
PPIB = C1 / A_I

# ---------------- problem geometry ----------------
B, C, T = 32, 40, 20000
NCORES = 8
NL = 160            # lanes per core
H = T // 2          # half length 10000
XL = H // 2         # extra-lane segment length 5000
P0 = 1              # first primary data col
PADX = 1 + H        # extra-region data1 boundary col
X0 = PADX + 1       # first extra data col
W = X0 + XL + 6     # tile width (padded)
ROUNDS_P = int(os.environ.get("NA_ROUNDS_P", "3"))   # primary-lane Gauss-Seidel rounds
ROUNDS_X = int(os.environ.get("NA_ROUNDS_X", "4"))   # extra-lane rounds (lag costs ~1 round)

F32 = mybir.dt.float32
MULT = mybir.AluOpType.mult
ADD = mybir.AluOpType.add

_CACHE = {}


def _build():
    nc = bacc.Bacc("TRN2", target_bir_lowering=False, debug=False,
                   num_devices=NCORES)
    x = nc.dram_tensor("x", [NL, T], F32, kind="ExternalInput").ap()
    y = nc.dram_tensor("y", [NL, T], F32, kind="ExternalOutput").ap()

    with TileContext(nc) as tc:
        emit_body(nc, tc, x, y)
    nc.compile()
    return nc


def emit_body(nc, tc, x, y):
    HC = H // 2         # primary column-chunk length (5000)
    if True:
        with tc.tile_pool(name="arr", bufs=1) as pool:
            A = pool.tile([128, W], F32)      # raw x -> softplus -> A -> ppi
            CI = pool.tile([128, W], F32)     # CI iterate -> d -> out
            CL = pool.tile([128, W], F32)     # CLb iterate (beta*CL); col 0 / PADX = data1 pads
            st = pool.tile([128, 8], F32)     # staging: 0 ci_init_p, 1 cl_init_p, 2 ci_init_x, 3 cl_init_x
            dc = pool.tile([128, 1], F32)     # delta broadcast source
            nc.gpsimd.memset(dc[:], DELTA)
            # per-half regions: (AP column start, length, partitions)
            REGS = [(P0, HC, 128), (P0 + HC, HC, 128), (X0, XL, 64)]

            for h in range(2):
                lo = h * H
                # ---- input DMA (chunked so ACT can start early) ----
                nc.sync.dma_start(out=A[:, P0:P0 + HC], in_=x[0:128, lo:lo + HC])
                nc.sync.dma_start(out=A[:, P0 + HC:P0 + H], in_=x[0:128, lo + HC:lo + H])
                nc.sync.dma_start(out=A[0:32, X0:X0 + XL], in_=x[128:160, lo:lo + XL])
                nc.sync.dma_start(out=A[32:64, X0:X0 + XL], in_=x[128:160, lo + XL:lo + H])
                # ---- softplus -> A (ACT), then A = C2*sp + C1 (GPSIMD) ----
                for (c0, ln, np_) in REGS:
                    nc.scalar.activation(A[0:np_, c0:c0 + ln], A[0:np_, c0:c0 + ln],
                                         mybir.ActivationFunctionType.Exp, scale=P1)
                    nc.scalar.activation(A[0:np_, c0:c0 + ln], A[0:np_, c0:c0 + ln],
                                         mybir.ActivationFunctionType.Ln, bias=1.0)
                    nc.gpsimd.tensor_scalar(A[0:np_, c0:c0 + ln], A[0:np_, c0:c0 + ln],
                                            C2, C1, MULT, ADD)
                if h == 0:
                    # t=0 columns: state must remain (CI0, CL0): zero A, inject via data1
                    nc.gpsimd.memset(A[:, P0:P0 + 1], 0.0)
                    nc.gpsimd.memset(A[0:32, X0:X0 + 1], 0.0)
                    # staging + data1 pads for h=0
                    nc.gpsimd.memset(st[:, 0:1], CI0)
                    nc.gpsimd.memset(st[:, 1:2], VINIT)
                    nc.gpsimd.memset(st[0:64, 2:3], CI0)
                    nc.gpsimd.memset(st[0:64, 3:4], VINIT)
                    nc.gpsimd.memset(CL[:, 0:1], CI0)          # primary data1 inject
                    nc.gpsimd.memset(CL[0:32, PADX:PADX + 1], CI0)
                    nc.gpsimd.memset(CL[32:64, PADX:PADX + 1], CL0B)

                for r in range(max(ROUNDS_P, ROUNDS_X) + 1):
                    # ---- CI scans (primary chunk A, chunk B chained, extra) ----
                    if r <= ROUNDS_P:
                        nc.vector.tensor_tensor_scan(
                            CI[:, P0:P0 + HC], A[:, P0:P0 + HC], CL[:, P0 - 1:P0 - 1 + HC],
                            st[:, 0:1], MULT, ADD)
                        nc.vector.tensor_tensor_scan(
                            CI[:, P0 + HC:P0 + H], A[:, P0 + HC:P0 + H],
                            CL[:, P0 + HC - 1:P0 + H - 1],
                            CI[:, P0 + HC - 1:P0 + HC], MULT, ADD)
                    if r <= ROUNDS_X:
                        nc.vector.tensor_tensor_scan(
                            CI[0:64, X0:X0 + XL], A[0:64, X0:X0 + XL],
                            CL[0:64, X0 - 1:X0 - 1 + XL], st[0:64, 2:3], MULT, ADD)
                    if r == max(ROUNDS_P, ROUNDS_X):
                        break
                    # lag-stage: seg0 end CI -> seg1 initial (next round)
                    if r < ROUNDS_X:
                        nc.sync.dma_start(out=st[32:64, 2:3], in_=CI[0:32, X0 + XL - 1:X0 + XL])
                    # ---- d-prep in place (ACT engine, overlaps later scans):  d = GB*CI + BC ----
                    for (c0, ln, np_) in REGS:
                        if (np_ == 128 and r < ROUNDS_P) or (np_ == 64 and r < ROUNDS_X):
                            nc.scalar.activation(CI[0:np_, c0:c0 + ln], CI[0:np_, c0:c0 + ln],
                                                 mybir.ActivationFunctionType.Copy,
                                                 scale=GB, bias=BC)
                    # ---- delta scans ----
                    if r < ROUNDS_P:
                        nc.vector.tensor_tensor_scan(
                            CL[:, P0:P0 + HC], dc[:].broadcast_to([128, HC]), CI[:, P0:P0 + HC],
                            st[:, 1:2], MULT, ADD)
                        nc.vector.tensor_tensor_scan(
                            CL[:, P0 + HC:P0 + H], dc[:].broadcast_to([128, HC]),
                            CI[:, P0 + HC:P0 + H], CL[:, P0 + HC - 1:P0 + HC], MULT, ADD)
                    if r < ROUNDS_X:
                        nc.vector.tensor_tensor_scan(
                            CL[0:64, X0:X0 + XL], dc[0:64, :].broadcast_to([64, XL]),
                            CI[0:64, X0:X0 + XL], st[0:64, 3:4], MULT, ADD)
                        # lag-stage: seg0 end CLb -> seg1 initial + seg1 data1 pad
                        nc.sync.dma_start(out=st[32:64, 3:4], in_=CL[0:32, X0 + XL - 1:X0 + XL])
                        nc.sync.dma_start(out=CL[32:64, PADX:PADX + 1], in_=CL[0:32, X0 + XL - 1:X0 + XL])

                if h == 0:
                    # save end-of-half states as next half's inits/pads
                    nc.sync.dma_start(out=st[:, 0:1], in_=CI[:, P0 + H - 1:P0 + H])
                    nc.sync.dma_start(out=st[0:32, 2:3], in_=CI[32:64, X0 + XL - 1:X0 + XL])
                    nc.sync.dma_start(out=st[32:64, 2:3], in_=CI[32:64, X0 + XL - 1:X0 + XL])
                    nc.sync.dma_start(out=st[:, 1:2], in_=CL[:, P0 + H - 1:P0 + H])
                    nc.sync.dma_start(out=CL[:, 0:1], in_=CL[:, P0 + H - 1:P0 + H])
                    nc.sync.dma_start(out=st[0:32, 3:4], in_=CL[32:64, X0 + XL - 1:X0 + XL])
                    nc.sync.dma_start(out=st[32:64, 3:4], in_=CL[32:64, X0 + XL - 1:X0 + XL])
                    nc.sync.dma_start(out=CL[0:32, PADX:PADX + 1], in_=CL[32:64, X0 + XL - 1:X0 + XL])
                    nc.sync.dma_start(out=CL[32:64, PADX:PADX + 1], in_=CL[32:64, X0 + XL - 1:X0 + XL])

                # ---- output: out = CI * ppi,  ppi = PPIA*A + PPIB (chunked, overlap DMA) ----
                for (c0, ln, np_) in REGS:
                    nc.gpsimd.tensor_scalar(A[0:np_, c0:c0 + ln], A[0:np_, c0:c0 + ln],
                                            PPIA, PPIB, MULT, ADD)
                    nc.gpsimd.tensor_tensor(CI[0:np_, c0:c0 + ln], CI[0:np_, c0:c0 + ln],
                                            A[0:np_, c0:c0 + ln], MULT)
                if h == 0:
                    nc.gpsimd.memset(CI[:, P0:P0 + 1], SPONT)
                    nc.gpsimd.memset(CI[0:32, X0:X0 + 1], SPONT)
                # ---- output DMA (chunked) ----
                nc.scalar.dma_start(out=y[0:128, lo:lo + HC], in_=CI[:, P0:P0 + HC])
                nc.scalar.dma_start(out=y[0:128, lo + HC:lo + H], in_=CI[:, P0 + HC:P0 + H])
                nc.scalar.dma_start(out=y[128:160, lo:lo + XL], in_=CI[0:32, X0:X0 + XL])
                nc.scalar.dma_start(out=y[128:160, lo + XL:lo + H], in_=CI[32:64, X0:X0 + XL])


def kernel(ihcl: np.ndarray) -> np.ndarray:
    assert ihcl.shape == (B, C, T) and ihcl.dtype == np.float32
    if "nc" not in _CACHE:
        _CACHE["nc"] = _build()
    nc = _CACHE["nc"]
    flat = np.ascontiguousarray(ihcl.reshape(B * C, T))
    in_maps = [{"x": flat[i * NL:(i + 1) * NL]} for i in range(NCORES)]
    res = run_bass_kernel_spmd(nc, in_maps, list(range(NCORES)))
    out = np.empty((B * C, T), np.float32)
    for i in range(NCORES):
        out[i * NL:(i + 1) * NL] = res.results[i]["y"]
    return out.reshape(B, C, T)
